# revision 20
# baseline (speedup 1.0000x reference)
"""Trainium2 Bass kernel for nn_Attention_msa (sparse cosine attention).

Head-sharded across 8 NeuronCores: core h computes head h of
  qkv = x @ W^T;  qn,kn,vn cosine-normalized
  attn = softmax((qn@kn^T) * 25 * cs[j] * mask[i,j])   mask = cs[j] > cs[i]-0.1
  x_h = attn @ v  ;  x_ori_h = v
  vv = vn@vn^T
The head-summed attn and vv go through a chunked fp16 ReduceScatter; each core
then finishes sim_round2 = renorm(mask(sim_raw>0.75) * softmax(mean_h attn))
for its slice of rows.

Layout notes:
  - Everything on-chip is kept in "transposed" [d, n] layout so the d=128
    head dim is the partition/contraction axis.
  - Host pre-transposes x (to x^T [C, N]) and the per-head W slices.
  - Outputs: out_xT [128, N] = (attn@v)^T, out_vT [128, N] = v^T,
    out_sim [N/8, N]. Host transposes/concats.
"""

import sys
import numpy as np

for _p in ("/opt/trn_rl_repo", "/root/.axon_site/_ro/trn_rl_repo"):
    if _p not in sys.path:
        sys.path.insert(0, _p)

import concourse.bass as bass
import concourse.tile as tile
from concourse import bacc, mybir
from concourse.masks import make_identity

F32 = mybir.dt.float32
F32R = mybir.dt.float32r
F16 = mybir.dt.float16
AF = mybir.ActivationFunctionType
ALU = mybir.AluOpType

N = 2048
C = 1024
H = 8
HD = 128
SCALE = 25.0
SIM_THRESH = 0.75
N_CORES = 8
N_BLOCKS = N // 128          # 16 i-blocks of 128 rows
N_RS = 4                     # number of chunked ReduceScatter calls
BG = N_BLOCKS // N_RS        # i-blocks per RS group
S_ROWS = 128 * BG // N_CORES  # rows per RS slot (64)
KC = C // 128                # contraction chunks for projections (8)
NF = N // 512                # 512-wide free-dim chunks (4)


GROUPS = [(0, 8), (8, 4), (12, 2), (14, 2)]  # (start, len) per RS group
F8 = mybir.dt.float8e4


def build_nc(fp32r_matmul=True):
    nc = bacc.Bacc("TRN2", target_bir_lowering=False, debug=False,
                   num_devices=N_CORES)

    xT_d = nc.dram_tensor("xT", [C, N], F32R, kind="ExternalInput")
    wT_d = nc.dram_tensor("wT", [C, 3 * HD], F32R, kind="ExternalInput")
    cs_d = nc.dram_tensor("cs", [N], F32, kind="ExternalInput")
    out_xT = nc.dram_tensor("out_xT", [HD, N], F32, kind="ExternalOutput")
    out_vT = nc.dram_tensor("out_vT", [HD, N], F32, kind="ExternalOutput")
    out_sim = nc.dram_tensor("out_sim", [N // N_CORES, N], F32,
                             kind="ExternalOutput")

    HN = N // 2  # half-width S-psum chunk

    from contextlib import ExitStack
    with tile.TileContext(nc) as tc, ExitStack() as ctx:
        singles = ctx.enter_context(tc.tile_pool(name="singles", bufs=1))
        io_ctx = tc.tile_pool(name="io", bufs=1)
        io = io_ctx.__enter__()
        ps_s = ctx.enter_context(tc.tile_pool(name="ps_s", bufs=2, space="PSUM"))
        ps_tp = ctx.enter_context(tc.tile_pool(name="ps_tp", bufs=2, space="PSUM"))
        ps_pv = ctx.enter_context(tc.tile_pool(name="ps_pv", bufs=2, space="PSUM"))
        dram = ctx.enter_context(tc.tile_pool(name="dram", bufs=1, space="DRAM"))
        rows_ctx = tc.tile_pool(name="rows", bufs=2)
        rows = rows_ctx.__enter__()
        sq_ctx = tc.tile_pool(name="sqpool", bufs=2)
        sqp = sq_ctx.__enter__()

        # ---- inputs: interleave w/x chunk DMAs so kc=0 lands first ----
        wT = io.tile([128, KC, 3 * HD], F32R, tag="wT")
        wTr = wT_d.ap().rearrange("(k p) m -> p k m", p=128)
        xT = io.tile([128, KC, N], F32R, tag="xT")
        xTr = xT_d.ap().rearrange("(k p) n -> p k n", p=128)
        for kc in range(KC):
            nc.sync.dma_start(out=wT[:, kc, :], in_=wTr[:, kc, :])
            nc.sync.dma_start(out=xT[:, kc, :], in_=xTr[:, kc, :])

        # small/constant DMAs ride the idle gpsimd (SWDGE) queue
        cs_b = singles.tile([128, N], F32, tag="cs_b")
        nc.gpsimd.dma_start(out=cs_b, in_=bass.AP(
            tensor=cs_d, offset=0, ap=[[0, 128], [1, N]]))
        cs_col = singles.tile([128, N_BLOCKS], F32, tag="cs_col")
        nc.gpsimd.dma_start(out=cs_col, in_=bass.AP(
            tensor=cs_d, offset=0, ap=[[1, 128], [128, N_BLOCKS]]))
        csm01 = singles.tile([128, N_BLOCKS], F32, tag="csm01")
        nc.vector.tensor_scalar_add(csm01, cs_col, -0.1)

        mask8 = singles.tile([128, N_BLOCKS, N], F8, tag="mask8")
        for b in range(N_BLOCKS):
            nc.vector.tensor_scalar(mask8[:, b, :], cs_b, csm01[:, b:b + 1],
                                    None, op0=ALU.is_gt)

        ident16 = singles.tile([128, 128], F16, tag="ident16")
        make_identity(nc, ident16)
        ones = singles.tile([128, 1], F32, tag="ones")
        nc.vector.memset(ones, 1.0)

        proj, bcast = {}, {}

        def do_proj(tname):
            tcol = {"q": 0, "k": 1, "v": 2}[tname]
            t_sb = singles.tile([128, N], F32R, tag=f"t_{tname}",
                                name=f"t_{tname}")
            for hc in range(2):
                psum = ps_s.tile([128, HN], F32, tag="s", name=f"ps_{tname}{hc}")
                for kc in range(KC):
                    lhsT = wT[:, kc, tcol * HD:(tcol + 1) * HD]
                    for nf in range(HN // 512):
                        o = nf * 512
                        nc.tensor.matmul(
                            psum[:, o:o + 512], lhsT,
                            xT[:, kc, hc * HN + o:hc * HN + o + 512],
                            start=(kc == 0), stop=(kc == KC - 1))
                nc.scalar.copy(t_sb[:, hc * HN:(hc + 1) * HN], psum)
            proj[tname] = t_sb
            return t_sb

        def do_norm_chain(tname):
            """sumsq over d -> broadcast raw row -> rsqrt on [128,N] ->
            in-place f32r column scale. DMAs on the gpsimd queue."""
            t_sb = proj[tname]
            sq = sqp.tile([128, N], F32, tag="sq", name=f"sq_{tname}")
            nc.vector.tensor_mul(sq, t_sb, t_sb)
            ssrow = rows.tile([1, N], F32, tag="row", name=f"ss_{tname}")
            for nf in range(NF):
                ssp = ps_tp.tile([1, 512], F32, tag="tp", name=f"ssp_{tname}{nf}")
                nc.tensor.matmul(ssp, ones, sq[:, nf * 512:(nf + 1) * 512],
                                 start=True, stop=True)
                nc.scalar.copy(ssrow[:, nf * 512:(nf + 1) * 512], ssp)
            dsc = dram.tile([N], F32, tag=f"dsc_{tname}", name=f"dsc_{tname}")
            nc.gpsimd.dma_start(out=dsc, in_=ssrow)
            bt = singles.tile([128, N], F32, tag=f"b_{tname}", name=f"b_{tname}")
            nc.gpsimd.dma_start(out=bt, in_=bass.AP(
                tensor=dsc.tensor, offset=dsc.offset, ap=[[0, 128], [1, N]]))
            nc.scalar.sqrt(bt, bt)
            nc.vector.reciprocal_approx_fast(bt, bt)
            if tname == "k":  # colfac[j] = cs[j]*inv_k[j]; SCALE rides the exp
                nc.vector.tensor_mul(bt, bt, cs_b)
            nc.vector.tensor_mul(t_sb, t_sb, bt)
            bcast[tname] = bt

        # ---- v first; raw-v consumers fire before the in-place scale ----
        do_proj("v")
        nc.gpsimd.dma_start(out=out_vT.ap(), in_=proj["v"].bitcast(F32))
        vT16 = singles.tile([128, N], F16, tag="vT16")
        nc.vector.tensor_copy(vT16, proj["v"])
        do_norm_chain("v")
        do_proj("k")
        do_norm_chain("k")
        do_proj("q")
        do_norm_chain("q")
        knT, vnT, qT = proj["k"], proj["v"], proj["q"]
        sq_ctx.__exit__(None, None, None)
        rows_ctx.__exit__(None, None, None)
        io_ctx.__exit__(None, None, None)
        work = ctx.enter_context(tc.tile_pool(name="work", bufs=2))

        # ---- ReduceScatter staging (fp8 payloads) ----
        rs_in_a, rs_in_v, rs_out_a, rs_out_v = [], [], [], []
        for g, (st, ln) in enumerate(GROUPS):
            s = 16 * ln
            rs_in_a.append(dram.tile([8, s, N], F8, tag=f"ra{g}", name=f"ra{g}"))
            rs_in_v.append(dram.tile([8, s, N], F8, tag=f"rv{g}", name=f"rv{g}"))
            rs_out_a.append(dram.tile([s, N], F8, tag=f"oa{g}", name=f"oa{g}"))
            rs_out_v.append(dram.tile([s, N], F8, tag=f"ov{g}", name=f"ov{g}"))

        def rs_pieces(b):
            for gi, (st, ln) in enumerate(GROUPS):
                if st <= b < st + ln:
                    s = 16 * ln
                    return [(gi, slice(p * s, (p + 1) * s),
                             ((b - st) * 128 + p * s) // s)
                            for p in range(128 // s)]
            raise AssertionError

        def group_end(b):
            return [gi for gi, (st, ln) in enumerate(GROUPS) if st + ln - 1 == b]


        # fp16 natural-layout v tiles for PV (from raw vT16)
        v16n = singles.tile([128, N_BLOCKS, 128], F16, tag="v16n")
        for g in range(N_BLOCKS // 4):
            tp = ps_tp.tile([128, 4, 128], F16, tag="tp", name=f"vtp{g}")
            for i in range(4):
                b = 4 * g + i
                nc.tensor.transpose(tp[:, i, :], vT16[:, b * 128:(b + 1) * 128],
                                    ident16)
            nc.vector.tensor_copy(v16n[:, 4 * g:4 * g + 4, :], tp)

        # ---- VV phase (between k and q so RS_v fires early and the q
        #      projection fills any VV pipeline gaps) ----
        for b in range(N_BLOCKS):
            vv8 = work.tile([128, N], F8, tag="vv8", bufs=10)
            for hc in range(2):
                psum = ps_s.tile([128, HN], F32, tag="s", name=f"vvps{b}_{hc}")
                for nf in range(HN // 512):
                    o = nf * 512
                    nc.tensor.matmul(psum[:, o:o + 512],
                                     vnT[:, b * 128:(b + 1) * 128],
                                     vnT[:, hc * HN + o:hc * HN + o + 512],
                                     start=True, stop=True)
                nc.scalar.copy(vv8[:, hc * HN:(hc + 1) * HN], psum)
            for gi, sl, c in rs_pieces(b):
                nc.gpsimd.dma_start(out=rs_in_v[gi][c], in_=vv8[sl, :])

        # ---- QK/softmax/PV phase ----
        for b in range(N_BLOCKS):
            xm = work.tile([128, N], F32, tag="xm")
            for hc in range(2):
                psum = ps_s.tile([128, HN], F32, tag="s", name=f"qkps{b}_{hc}")
                for nf in range(HN // 512):
                    o = nf * 512
                    nc.tensor.matmul(psum[:, o:o + 512],
                                     qT[:, b * 128:(b + 1) * 128],
                                     knT[:, hc * HN + o:hc * HN + o + 512],
                                     start=True, stop=True)
                nc.vector.tensor_tensor(xm[:, hc * HN:(hc + 1) * HN], psum,
                                        mask8[:, b, hc * HN:(hc + 1) * HN],
                                        op=ALU.mult)
            attn_un = work.tile([128, N], F32, tag="attn_un")
            rowsum = work.tile([128, 1], F32, tag="rowsum")
            nc.scalar.activation(attn_un, xm, AF.Exp, scale=float(SCALE),
                                 accum_out=rowsum)
            inv_rs = work.tile([128, 1], F32, tag="inv_rs")
            nc.vector.reciprocal_approx_fast(inv_rs, rowsum)
            attn16 = work.tile([128, N], F16, tag="attn16", bufs=3)
            nc.vector.tensor_scalar(attn16, attn_un, inv_rs, None, op0=ALU.mult)
            attn8 = work.tile([128, N], F8, tag="attn8", bufs=10)
            nc.vector.tensor_copy(attn8, attn16)
            for gi, sl, c in rs_pieces(b):
                nc.gpsimd.dma_start(out=rs_in_a[gi][c], in_=attn8[sl, :])
            # PV: 4-wide transpose PSUM tiles, one DVE copy per 4 transposes
            attnT = work.tile([128, N_BLOCKS, 128], F16, tag="attnT", bufs=3)
            for g in range(N_BLOCKS // 4):
                tp = ps_tp.tile([128, 4, 128], F16, tag="tp", name=f"atp{b}_{g}")
                for i in range(4):
                    j = 4 * g + i
                    nc.tensor.transpose(tp[:, i, :],
                                        attn16[:, j * 128:(j + 1) * 128],
                                        ident16)
                nc.vector.tensor_copy(attnT[:, 4 * g:4 * g + 4, :], tp)
            pv = ps_pv.tile([128, 128], F32, tag="pv")
            for j in range(N_BLOCKS):
                nc.tensor.matmul(pv, v16n[:, j, :], attnT[:, j, :],
                                 start=(j == 0), stop=(j == N_BLOCKS - 1))
            xout = work.tile([128, 128], F32, tag="xout", bufs=8)
            nc.vector.tensor_copy(xout, pv)
            nc.gpsimd.dma_start(out=out_xT.ap()[:, b * 128:(b + 1) * 128],
                                in_=xout)
            vv_rs = {1: 0, 4: 1, 7: 2, 10: 3}.get(b)
            if vv_rs is not None:
                nc.gpsimd.collective_compute(
                    "ReduceScatter", ALU.add,
                    replica_groups=[list(range(N_CORES))],
                    ins=[rs_in_v[vv_rs].opt()], outs=[rs_out_v[vv_rs].opt()])
            for gi in group_end(b):
                nc.gpsimd.collective_compute(
                    "ReduceScatter", ALU.add,
                    replica_groups=[list(range(N_CORES))],
                    ins=[rs_in_a[gi].opt()], outs=[rs_out_a[gi].opt()])

        # ---- final sim chain per group ----
        off = 0
        for gi, (st, ln) in enumerate(GROUPS):
            s = 16 * ln
            ta = work.tile([s, N], F8, tag="attn8", bufs=10, name=f"fin_a{gi}")
            nc.gpsimd.dma_start(out=ta, in_=rs_out_a[gi])
            tv = work.tile([s, N], F8, tag="vv8", bufs=10, name=f"fin_v{gi}")
            nc.gpsimd.dma_start(out=tv, in_=rs_out_v[gi])
            e = work.tile([s, N], F32, tag="xm", name=f"fin_e{gi}")
            nc.scalar.activation(e, ta, AF.Exp, scale=1.0 / H)
            m2 = work.tile([s, N], F32, tag="attn_un", name=f"fin_m{gi}")
            nc.vector.tensor_scalar(m2, tv, float(SIM_THRESH * H), None,
                                    op0=ALU.is_gt)
            nc.vector.tensor_mul(e, e, m2)
            msum = work.tile([s, 1], F32, tag="rowsum", name=f"fin_ms{gi}")
            nc.vector.reduce_sum(msum, e, axis=mybir.AxisListType.X)
            minv = work.tile([s, 1], F32, tag="inv_rs", name=f"fin_mi{gi}")
            nc.vector.reciprocal_approx_fast(minv, msum)
            outt = work.tile([s, N], F32, tag="attn_un", name=f"fin_out{gi}")
            nc.vector.tensor_scalar(outt, e, minv, None, op0=ALU.mult)
            nc.gpsimd.dma_start(out=out_sim.ap()[off:off + s, :], in_=outt)
            off += s

    nc.compile()
    return nc


_NC_CACHE = {}


def tf32_round(a):
    u = np.ascontiguousarray(a, dtype=np.float32).view(np.uint32)
    r = (u + np.uint32(0x1000) + ((u >> np.uint32(13)) & np.uint32(1))) \
        & ~np.uint32(0x1FFF)
    return r.view(np.float32)


def kernel(x_cls, cls_score, fg_score, W_qkv):
    from concourse.bass_utils import run_bass_kernel_spmd

    x_cls = np.asarray(x_cls)
    cls_score = np.asarray(cls_score, dtype=np.float32)
    W_qkv = np.asarray(W_qkv, dtype=np.float32)
    B = x_cls.shape[0]
    xT = tf32_round(np.ascontiguousarray(x_cls.reshape(N, C).T.astype(np.float32)))

    if "nc" not in _NC_CACHE:
        _NC_CACHE["nc"] = build_nc()
    nc = _NC_CACHE["nc"]

    in_maps = []
    for h in range(N_CORES):
        w_h = np.concatenate([
            W_qkv[0 * C + h * HD:0 * C + (h + 1) * HD],   # q rows [HD, C]
            W_qkv[1 * C + h * HD:1 * C + (h + 1) * HD],   # k rows
            W_qkv[2 * C + h * HD:2 * C + (h + 1) * HD],   # v rows
        ], axis=0)                                        # [3HD, C]
        wT_h = tf32_round(np.ascontiguousarray(w_h.T))    # [C, 3HD]
        in_maps.append({"xT": xT, "wT": wT_h, "cs": cls_score})

    res = run_bass_kernel_spmd(nc, in_maps, list(range(N_CORES)))
    outs = res.results

    x = np.empty((N, C), np.float32)
    x_ori = np.empty((N, C), np.float32)
    sim = np.empty((N, N), np.float32)
    for h in range(N_CORES):
        x[:, h * HD:(h + 1) * HD] = outs[h]["out_xT"].T
        x_ori[:, h * HD:(h + 1) * HD] = outs[h]["out_vT"].T
        os = outs[h]["out_sim"]                           # [N/8, N]
        off = 0
        for (st, ln) in GROUPS:
            s = 16 * ln
            r0 = 128 * st + s * h
            sim[r0:r0 + s] = os[off:off + s]
            off += s
    x_out = np.concatenate([x, x_ori], axis=-1).reshape(B, N, 2 * C)
    return x_out, sim


# revision 21
# speedup vs baseline: 1.1185x; 1.1185x over previous
"""Trainium2 Bass kernel for nn_Attention_msa (sparse cosine attention).

Head-sharded across 8 NeuronCores: core h computes head h of
  qkv = x @ W^T;  qn,kn,vn cosine-normalized
  attn = softmax((qn@kn^T) * 25 * cs[j] * mask[i,j])   mask = cs[j] > cs[i]-0.1
  x_h = attn @ v  ;  x_ori_h = v
  vv = vn@vn^T
The head-summed attn and vv go through a chunked fp16 ReduceScatter; each core
then finishes sim_round2 = renorm(mask(sim_raw>0.75) * softmax(mean_h attn))
for its slice of rows.

Layout notes:
  - Everything on-chip is kept in "transposed" [d, n] layout so the d=128
    head dim is the partition/contraction axis.
  - Host pre-transposes x (to x^T [C, N]) and the per-head W slices.
  - Outputs: out_xT [128, N] = (attn@v)^T, out_vT [128, N] = v^T,
    out_sim [N/8, N]. Host transposes/concats.
"""

import sys
import numpy as np

for _p in ("/opt/trn_rl_repo", "/root/.axon_site/_ro/trn_rl_repo"):
    if _p not in sys.path:
        sys.path.insert(0, _p)

import concourse.bass as bass
import concourse.tile as tile
from concourse import bacc, mybir
from concourse.masks import make_identity

F32 = mybir.dt.float32
F32R = mybir.dt.float32r
F16 = mybir.dt.float16
AF = mybir.ActivationFunctionType
ALU = mybir.AluOpType

N = 2048
C = 1024
H = 8
HD = 128
SCALE = 25.0
SIM_THRESH = 0.75
N_CORES = 8
N_BLOCKS = N // 128          # 16 i-blocks of 128 rows
N_RS = 4                     # number of chunked ReduceScatter calls
BG = N_BLOCKS // N_RS        # i-blocks per RS group
S_ROWS = 128 * BG // N_CORES  # rows per RS slot (64)
KC = C // 128                # contraction chunks for projections (8)
NF = N // 512                # 512-wide free-dim chunks (4)


GROUPS = [(0, 8), (8, 4), (12, 4)]  # (start, len) per RS group
F8 = mybir.dt.float8e4


def build_nc(fp32r_matmul=True):
    nc = bacc.Bacc("TRN2", target_bir_lowering=False, debug=False,
                   num_devices=N_CORES)

    xT_d = nc.dram_tensor("xT", [C, N], F32R, kind="ExternalInput")
    wT_d = nc.dram_tensor("wT", [C, 3 * HD], F32R, kind="ExternalInput")
    cs_d = nc.dram_tensor("cs", [N], F32, kind="ExternalInput")
    out_xT = nc.dram_tensor("out_xT", [HD, N], F32, kind="ExternalOutput")
    out_vT = nc.dram_tensor("out_vT", [HD, N], F32, kind="ExternalOutput")
    out_sim = nc.dram_tensor("out_sim", [N // N_CORES, N], F32,
                             kind="ExternalOutput")

    HN = N // 2  # half-width S-psum chunk

    from contextlib import ExitStack
    with tile.TileContext(nc) as tc, ExitStack() as ctx:
        singles = ctx.enter_context(tc.tile_pool(name="singles", bufs=1))
        io_ctx = tc.tile_pool(name="io", bufs=1)
        io = io_ctx.__enter__()
        ps_s = ctx.enter_context(tc.tile_pool(name="ps_s", bufs=2, space="PSUM"))
        ps_tp = ctx.enter_context(tc.tile_pool(name="ps_tp", bufs=2, space="PSUM"))
        ps_pv = ctx.enter_context(tc.tile_pool(name="ps_pv", bufs=2, space="PSUM"))
        dram = ctx.enter_context(tc.tile_pool(name="dram", bufs=1, space="DRAM"))
        rows_ctx = tc.tile_pool(name="rows", bufs=2)
        rows = rows_ctx.__enter__()
        sq_ctx = tc.tile_pool(name="sqpool", bufs=2)
        sqp = sq_ctx.__enter__()

        # ---- inputs: interleave w/x chunk DMAs so kc=0 lands first ----
        wT = io.tile([128, KC, 3 * HD], F32R, tag="wT")
        wTr = wT_d.ap().rearrange("(k p) m -> p k m", p=128)
        xT = io.tile([128, KC, N], F32R, tag="xT")
        xTr = xT_d.ap().rearrange("(k p) n -> p k n", p=128)
        for kc in range(KC):
            nc.sync.dma_start(out=wT[:, kc, :], in_=wTr[:, kc, :])
            nc.sync.dma_start(out=xT[:, kc, :], in_=xTr[:, kc, :])

        # small/constant DMAs ride the idle gpsimd (SWDGE) queue
        cs_b = singles.tile([128, N], F32, tag="cs_b")
        nc.gpsimd.dma_start(out=cs_b, in_=bass.AP(
            tensor=cs_d, offset=0, ap=[[0, 128], [1, N]]))
        cs_col = singles.tile([128, N_BLOCKS], F32, tag="cs_col")
        nc.gpsimd.dma_start(out=cs_col, in_=bass.AP(
            tensor=cs_d, offset=0, ap=[[1, 128], [128, N_BLOCKS]]))
        csm01 = singles.tile([128, N_BLOCKS], F32, tag="csm01")
        nc.vector.tensor_scalar_add(csm01, cs_col, -0.1)

        mask8 = singles.tile([128, N_BLOCKS, N], F8, tag="mask8")
        for b in range(N_BLOCKS):
            nc.vector.tensor_scalar(mask8[:, b, :], cs_b, csm01[:, b:b + 1],
                                    None, op0=ALU.is_gt)

        ident16 = singles.tile([128, 128], F16, tag="ident16")
        make_identity(nc, ident16)
        ones = singles.tile([128, 1], F32, tag="ones")
        nc.vector.memset(ones, 1.0)

        proj, bcast = {}, {}

        def do_proj(tname):
            tcol = {"q": 0, "k": 1, "v": 2}[tname]
            t_sb = singles.tile([128, N], F32R, tag=f"t_{tname}",
                                name=f"t_{tname}")
            for hc in range(2):
                psum = ps_s.tile([128, HN], F32, tag="s", name=f"ps_{tname}{hc}")
                for kc in range(KC):
                    lhsT = wT[:, kc, tcol * HD:(tcol + 1) * HD]
                    for nf in range(HN // 512):
                        o = nf * 512
                        nc.tensor.matmul(
                            psum[:, o:o + 512], lhsT,
                            xT[:, kc, hc * HN + o:hc * HN + o + 512],
                            start=(kc == 0), stop=(kc == KC - 1))
                nc.scalar.copy(t_sb[:, hc * HN:(hc + 1) * HN], psum)
            proj[tname] = t_sb
            return t_sb

        def do_norm_chain(tname):
            """sumsq over d -> broadcast raw row -> rsqrt on [128,N] ->
            in-place f32r column scale. DMAs on the gpsimd queue."""
            t_sb = proj[tname]
            sq = sqp.tile([128, N], F32, tag="sq", name=f"sq_{tname}")
            nc.vector.tensor_mul(sq, t_sb, t_sb)
            ssrow = rows.tile([1, N], F32, tag="row", name=f"ss_{tname}")
            for nf in range(NF):
                ssp = ps_tp.tile([1, 512], F32, tag="tp", name=f"ssp_{tname}{nf}")
                nc.tensor.matmul(ssp, ones, sq[:, nf * 512:(nf + 1) * 512],
                                 start=True, stop=True)
                nc.scalar.copy(ssrow[:, nf * 512:(nf + 1) * 512], ssp)
            dsc = dram.tile([N], F32, tag=f"dsc_{tname}", name=f"dsc_{tname}")
            nc.gpsimd.dma_start(out=dsc, in_=ssrow)
            bt = singles.tile([128, N], F32, tag=f"b_{tname}", name=f"b_{tname}")
            nc.gpsimd.dma_start(out=bt, in_=bass.AP(
                tensor=dsc.tensor, offset=dsc.offset, ap=[[0, 128], [1, N]]))
            nc.scalar.sqrt(bt, bt)
            nc.vector.reciprocal_approx_fast(bt, bt)
            if tname == "k":  # colfac[j] = cs[j]*inv_k[j]; SCALE rides the exp
                nc.vector.tensor_mul(bt, bt, cs_b)
            nc.vector.tensor_mul(t_sb, t_sb, bt)
            bcast[tname] = bt

        # ---- v first; raw-v consumers fire before the in-place scale ----
        do_proj("v")
        nc.gpsimd.dma_start(out=out_vT.ap(), in_=proj["v"].bitcast(F32))
        vT16 = singles.tile([128, N], F16, tag="vT16")
        nc.vector.tensor_copy(vT16, proj["v"])
        do_norm_chain("v")
        do_proj("k")
        do_norm_chain("k")
        do_proj("q")
        do_norm_chain("q")
        knT, vnT, qT = proj["k"], proj["v"], proj["q"]
        sq_ctx.__exit__(None, None, None)
        rows_ctx.__exit__(None, None, None)
        io_ctx.__exit__(None, None, None)
        work = ctx.enter_context(tc.tile_pool(name="work", bufs=2))

        # ---- ReduceScatter staging (fp8 payloads) ----
        rs_in_a, rs_in_v, rs_out_a, rs_out_v = [], [], [], []
        for g, (st, ln) in enumerate(GROUPS):
            s = 16 * ln
            rs_in_a.append(dram.tile([8, s, N], F8, tag=f"ra{g}", name=f"ra{g}"))
            rs_in_v.append(dram.tile([8, s, N], F8, tag=f"rv{g}", name=f"rv{g}"))
            rs_out_a.append(dram.tile([s, N], F8, tag=f"oa{g}", name=f"oa{g}"))
            rs_out_v.append(dram.tile([s, N], F8, tag=f"ov{g}", name=f"ov{g}"))

        def rs_pieces(b):
            for gi, (st, ln) in enumerate(GROUPS):
                if st <= b < st + ln:
                    s = 16 * ln
                    return [(gi, slice(p * s, (p + 1) * s),
                             ((b - st) * 128 + p * s) // s)
                            for p in range(128 // s)]
            raise AssertionError

        def group_end(b):
            return [gi for gi, (st, ln) in enumerate(GROUPS) if st + ln - 1 == b]


        # fp16 natural-layout v tiles for PV (from raw vT16)
        v16n = singles.tile([128, N_BLOCKS, 128], F16, tag="v16n")
        for g in range(N_BLOCKS // 4):
            tp = ps_tp.tile([128, 4, 128], F16, tag="tp", name=f"vtp{g}")
            for i in range(4):
                b = 4 * g + i
                nc.tensor.transpose(tp[:, i, :], vT16[:, b * 128:(b + 1) * 128],
                                    ident16)
            nc.vector.tensor_copy(v16n[:, 4 * g:4 * g + 4, :], tp)

        # ---- VV phase (between k and q so RS_v fires early and the q
        #      projection fills any VV pipeline gaps) ----
        for b in range(N_BLOCKS):
            vv8 = work.tile([128, N], F8, tag="vv8", bufs=10)
            for hc in range(2):
                psum = ps_s.tile([128, HN], F32, tag="s", name=f"vvps{b}_{hc}")
                for nf in range(HN // 512):
                    o = nf * 512
                    nc.tensor.matmul(psum[:, o:o + 512],
                                     vnT[:, b * 128:(b + 1) * 128],
                                     vnT[:, hc * HN + o:hc * HN + o + 512],
                                     start=True, stop=True)
                nc.scalar.copy(vv8[:, hc * HN:(hc + 1) * HN], psum)
            for gi, sl, c in rs_pieces(b):
                nc.sync.dma_start(out=rs_in_v[gi][c], in_=vv8[sl, :])

        # ---- QK/softmax/PV phase ----
        for b in range(N_BLOCKS):
            xm = work.tile([128, N], F32, tag="xm")
            for hc in range(2):
                psum = ps_s.tile([128, HN], F32, tag="s", name=f"qkps{b}_{hc}")
                for nf in range(HN // 512):
                    o = nf * 512
                    nc.tensor.matmul(psum[:, o:o + 512],
                                     qT[:, b * 128:(b + 1) * 128],
                                     knT[:, hc * HN + o:hc * HN + o + 512],
                                     start=True, stop=True)
                nc.vector.tensor_tensor(xm[:, hc * HN:(hc + 1) * HN], psum,
                                        mask8[:, b, hc * HN:(hc + 1) * HN],
                                        op=ALU.mult)
            attn_un = work.tile([128, N], F32, tag="attn_un")
            rowsum = work.tile([128, 1], F32, tag="rowsum")
            nc.scalar.activation(attn_un, xm, AF.Exp, scale=float(SCALE),
                                 accum_out=rowsum)
            inv_rs = work.tile([128, 1], F32, tag="inv_rs")
            nc.vector.reciprocal_approx_fast(inv_rs, rowsum)
            attn16 = work.tile([128, N], F16, tag="attn16", bufs=3)
            nc.vector.tensor_scalar(attn16, attn_un, inv_rs, None, op0=ALU.mult)
            attn8 = work.tile([128, N], F8, tag="attn8", bufs=10)
            nc.vector.tensor_copy(attn8, attn16)
            for gi, sl, c in rs_pieces(b):
                nc.sync.dma_start(out=rs_in_a[gi][c], in_=attn8[sl, :])
            # PV: 4-wide transpose PSUM tiles, one DVE copy per 4 transposes
            attnT = work.tile([128, N_BLOCKS, 128], F16, tag="attnT", bufs=3)
            for g in range(N_BLOCKS // 4):
                tp = ps_tp.tile([128, 4, 128], F16, tag="tp", name=f"atp{b}_{g}")
                for i in range(4):
                    j = 4 * g + i
                    nc.tensor.transpose(tp[:, i, :],
                                        attn16[:, j * 128:(j + 1) * 128],
                                        ident16)
                nc.vector.tensor_copy(attnT[:, 4 * g:4 * g + 4, :], tp)
            pv = ps_pv.tile([128, 128], F32, tag="pv")
            for j in range(N_BLOCKS):
                nc.tensor.matmul(pv, v16n[:, j, :], attnT[:, j, :],
                                 start=(j == 0), stop=(j == N_BLOCKS - 1))
            xout = work.tile([128, 128], F32, tag="xout", bufs=8)
            nc.vector.tensor_copy(xout, pv)
            nc.sync.dma_start(out=out_xT.ap()[:, b * 128:(b + 1) * 128],
                              in_=xout)
            vv_rs = {1: 0, 4: 1, 7: 2}.get(b)
            if vv_rs is not None:
                nc.gpsimd.collective_compute(
                    "ReduceScatter", ALU.add,
                    replica_groups=[list(range(N_CORES))],
                    ins=[rs_in_v[vv_rs].opt()], outs=[rs_out_v[vv_rs].opt()])
            for gi in group_end(b):
                nc.gpsimd.collective_compute(
                    "ReduceScatter", ALU.add,
                    replica_groups=[list(range(N_CORES))],
                    ins=[rs_in_a[gi].opt()], outs=[rs_out_a[gi].opt()])

        # ---- final sim chain per group ----
        off = 0
        for gi, (st, ln) in enumerate(GROUPS):
            s = 16 * ln
            ta = work.tile([s, N], F8, tag="attn8", bufs=10, name=f"fin_a{gi}")
            nc.sync.dma_start(out=ta, in_=rs_out_a[gi])
            tv = work.tile([s, N], F8, tag="vv8", bufs=10, name=f"fin_v{gi}")
            nc.sync.dma_start(out=tv, in_=rs_out_v[gi])
            e = work.tile([s, N], F32, tag="xm", name=f"fin_e{gi}")
            nc.scalar.activation(e, ta, AF.Exp, scale=1.0 / H)
            m2 = work.tile([s, N], F32, tag="attn_un", name=f"fin_m{gi}")
            nc.vector.tensor_scalar(m2, tv, float(SIM_THRESH * H), None,
                                    op0=ALU.is_gt)
            nc.vector.tensor_mul(e, e, m2)
            msum = work.tile([s, 1], F32, tag="rowsum", name=f"fin_ms{gi}")
            nc.vector.reduce_sum(msum, e, axis=mybir.AxisListType.X)
            minv = work.tile([s, 1], F32, tag="inv_rs", name=f"fin_mi{gi}")
            nc.vector.reciprocal_approx_fast(minv, msum)
            outt = work.tile([s, N], F32, tag="attn_un", name=f"fin_out{gi}")
            nc.vector.tensor_scalar(outt, e, minv, None, op0=ALU.mult)
            nc.sync.dma_start(out=out_sim.ap()[off:off + s, :], in_=outt)
            off += s

    nc.compile()
    return nc


_NC_CACHE = {}


def tf32_round(a):
    u = np.ascontiguousarray(a, dtype=np.float32).view(np.uint32)
    r = (u + np.uint32(0x1000) + ((u >> np.uint32(13)) & np.uint32(1))) \
        & ~np.uint32(0x1FFF)
    return r.view(np.float32)


def kernel(x_cls, cls_score, fg_score, W_qkv):
    from concourse.bass_utils import run_bass_kernel_spmd

    x_cls = np.asarray(x_cls)
    cls_score = np.asarray(cls_score, dtype=np.float32)
    W_qkv = np.asarray(W_qkv, dtype=np.float32)
    B = x_cls.shape[0]
    xT = tf32_round(np.ascontiguousarray(x_cls.reshape(N, C).T.astype(np.float32)))

    if "nc" not in _NC_CACHE:
        _NC_CACHE["nc"] = build_nc()
    nc = _NC_CACHE["nc"]

    in_maps = []
    for h in range(N_CORES):
        w_h = np.concatenate([
            W_qkv[0 * C + h * HD:0 * C + (h + 1) * HD],   # q rows [HD, C]
            W_qkv[1 * C + h * HD:1 * C + (h + 1) * HD],   # k rows
            W_qkv[2 * C + h * HD:2 * C + (h + 1) * HD],   # v rows
        ], axis=0)                                        # [3HD, C]
        wT_h = tf32_round(np.ascontiguousarray(w_h.T))    # [C, 3HD]
        in_maps.append({"xT": xT, "wT": wT_h, "cs": cls_score})

    res = run_bass_kernel_spmd(nc, in_maps, list(range(N_CORES)))
    outs = res.results

    x = np.empty((N, C), np.float32)
    x_ori = np.empty((N, C), np.float32)
    sim = np.empty((N, N), np.float32)
    for h in range(N_CORES):
        x[:, h * HD:(h + 1) * HD] = outs[h]["out_xT"].T
        x_ori[:, h * HD:(h + 1) * HD] = outs[h]["out_vT"].T
        os = outs[h]["out_sim"]                           # [N/8, N]
        off = 0
        for (st, ln) in GROUPS:
            s = 16 * ln
            r0 = 128 * st + s * h
            sim[r0:r0 + s] = os[off:off + s]
            off += s
    x_out = np.concatenate([x, x_ori], axis=-1).reshape(B, N, 2 * C)
    return x_out, sim
